# revision 7
# baseline (speedup 1.0000x reference)
"""GCN layer (gather -> weighted scatter-sum -> dense transform) on 8 trn2 cores.

Strategy (1-D row partitioning of destination nodes), v2:
  - Core c owns destination nodes [c*NPW, (c+1)*NPW). edge_dst is sorted, so
    each core's edges are a contiguous slice of the edge list.
  - Dst nodes are processed in windows of 128 (PSUM partition size). Each core
    processes ITS windows in DESCENDING edge-count order ("slots"), so the
    shared SPMD per-slot budgets (max over cores) track the per-core sorted
    quantiles and padding stays ~2%. The host unpermutes output columns.
  - Gather indices are int16 for SWDGE dma_gather. Instead of a hard lo/hi
    split at 32768 (which pads two streams separately), we gather from two
    OVERLAPPING views of H: image A = rows [0, 32768), image B = rows
    [17232, 50000) (both int16-addressable). Edges with src in the overlap
    [17232, 32768) are flexible and are assigned per-core to fill image-A
    groups exactly, so each slot needs only ceil(max_edges/128) total groups
    (single rounding).
  - Per 128-edge group: TensorE accumulates aggT[feat, dst] += G.T @ S in
    PSUM, where G = dma_gather'd source rows (fp16) and S[e, dst] =
    (iota[dst] == drel[e]) * w[e] built by ONE fused DVE tensor_scalar per
    group (two per-partition scalar operands) -- half the DVE traffic of the
    two-pass tensor_tensor build, which also reduces SBUF-port contention
    with the Q7 SWDGE descriptor writes.
  - The final transform out.T = W.T @ aggT (+b) is interleaved into the
    window loop (one 512-col chunk per 4 finished slots) so it hides under
    the gather stream instead of adding tail latency.
  - dma_gather calls are chunks of <=8 groups (8k+1 SWDGE ring entries;
    k<=8 proven safe on HW, k=12 crashes the exec unit).
"""

import os
import numpy as np

N_CORES = 8
N_NODES = 50000
D = 128
NPW = N_NODES // N_CORES  # 6250 dst nodes per core
WIN = 128
IMG_A_ROWS = 32768       # image A = H[0:32768]
CUT_B = 32768            # src >= CUT_B must use image B
IMG_B_BASE = N_NODES - 32768  # 17232; image B = H[17232:50000]
MAXG = 8                 # max groups (128 idx each) per dma_gather call

GDTYPE = os.environ.get("GCN_GDTYPE", "f16")

LAST_EXEC_NS = None
LAST_RESULTS = None


def _ceil_div(a, b):
    return -(-a // b)


def _prep(edge_src, edge_dst):
    """Host-side plan: per-core slot-ordered windows, shared slot budgets,
    per-core packed edge lists (A-image then B-image groups)."""
    nwin = _ceil_div(NPW, WIN)
    cores = []
    cnt = np.zeros((N_CORES, nwin), np.int64)
    cntA = np.zeros((N_CORES, nwin), np.int64)  # must-A (src < IMG_B_BASE)
    cntB = np.zeros((N_CORES, nwin), np.int64)  # must-B (src >= CUT_B)
    for c in range(N_CORES):
        e0, e1 = np.searchsorted(edge_dst, [c * NPW, (c + 1) * NPW])
        d = edge_dst[e0:e1] - c * NPW
        s = edge_src[e0:e1]
        bounds = [np.searchsorted(d, wi * WIN) for wi in range(nwin + 1)]
        wins = []
        for wi in range(nwin):
            i0, i1 = bounds[wi], bounds[wi + 1]
            wins.append((s[i0:i1], d[i0:i1] - wi * WIN, (e0 + i0, e0 + i1)))
            cnt[c, wi] = i1 - i0
            cntA[c, wi] = int((s[i0:i1] < IMG_B_BASE).sum())
            cntB[c, wi] = int((s[i0:i1] >= CUT_B).sum())
        cores.append(wins)

    order = np.argsort(-cnt, axis=1)  # per-core slot -> window
    cs = np.take_along_axis(cnt, order, 1)
    As = np.take_along_axis(cntA, order, 1)
    Bs = np.take_along_axis(cntB, order, 1)

    g_slot = np.maximum(
        _ceil_div(cs.max(0), 128),
        _ceil_div(As.max(0), 128) + _ceil_div(Bs.max(0), 128),
    ).astype(int)
    g_slot = np.maximum(g_slot, 1)
    gA_lo = np.maximum(_ceil_div(As.max(0), 128), 0).astype(int)
    gA_hi = (g_slot - np.maximum(_ceil_div(Bs.max(0), 128), 0)).astype(int)
    # prefer gA a multiple of MAXG (min call count), else low end
    gA_slot = np.empty(nwin, int)
    for i in range(nwin):
        lo, hi = int(gA_lo[i]), int(gA_hi[i])
        lo = max(lo, 0)
        hi = max(hi, lo)
        mult = _ceil_div(lo, MAXG) * MAXG
        gA_slot[i] = mult if lo <= mult <= hi else lo
    gB_slot = g_slot - gA_slot
    return cores, order, nwin, g_slot, gA_slot, gB_slot


def _chunks(g):
    out = []
    c0 = 0
    while c0 < g:
        k = min(MAXG, g - c0)
        out.append((c0, k))
        c0 += k
    return out


def _pack_core(wins, order_c, nwin, g_slot, gA_slot, gB_slot, edge_weight,
               np_g):
    np_m = np.float32  # metadata dtype (tensor_scalar wants fp32 scalars)
    """Build device arrays for one core.

    idx: per-call wrapped-16 blocks, concatenated: [16, sum(call 8*ke)] -> tiled
         to [128, .].
    drel/wgt: [128, sum(g_slot)] with column (slot base + group) and row p =
         edge at group position p."""
    tot_g = int(g_slot.sum())
    drel = np.zeros((128, tot_g), np_m)
    wgt = np.zeros((128, tot_g), np_m)
    idx_blocks = []
    gbase = 0
    for slot in range(nwin):
        wi = int(order_c[slot])
        s, d, (ge0, ge1) = wins[wi]
        w = edge_weight[ge0:ge1]
        gA, gB, g = int(gA_slot[slot]), int(gB_slot[slot]), int(g_slot[slot])
        capA = 128 * gA
        isB_forced = s >= CUT_B
        isA_forced = s < IMG_B_BASE
        nA0 = int(isA_forced.sum())
        # fill A with forced-A then flex until capA
        flex = ~isA_forced & ~isB_forced
        takeA_flex = min(max(capA - nA0, 0), int(flex.sum()))
        # order: forced-A edges, then takeA_flex flex edges -> image A
        idxA = np.flatnonzero(isA_forced)
        idxF = np.flatnonzero(flex)
        idxB = np.flatnonzero(isB_forced)
        selA = np.concatenate([idxA, idxF[:takeA_flex]])
        selB = np.concatenate([idxF[takeA_flex:], idxB])
        assert len(selA) <= 128 * gA and len(selB) <= 128 * gB, (
            slot, len(selA), gA, len(selB), gB)
        iA = np.zeros(128 * gA, np.int16)
        iB = np.zeros(128 * gB, np.int16)
        iA[: len(selA)] = s[selA].astype(np.int16)
        iB[: len(selB)] = (s[selB] - IMG_B_BASE).astype(np.int16)
        # metadata per group-column; A groups then B groups
        dd = np.zeros(128 * g, np_m)
        ww = np.zeros(128 * g, np_m)
        dd[: len(selA)] = d[selA].astype(np_m)
        ww[: len(selA)] = w[selA].astype(np_g).astype(np_m)
        dd[128 * gA : 128 * gA + len(selB)] = d[selB].astype(np_m)
        ww[128 * gA : 128 * gA + len(selB)] = w[selB].astype(np_g).astype(np_m)
        drel[:, gbase : gbase + g] = dd.reshape(g, 128).T
        wgt[:, gbase : gbase + g] = ww.reshape(g, 128).T
        # idx blocks per call (A calls then B calls), wrapped 16
        for (c0, k) in _chunks(gA):
            flat = iA[c0 * 128 : (c0 + k) * 128]
            idx_blocks.append(flat.reshape(-1, 16).T)
        for (c0, k) in _chunks(gB):
            flat = iB[c0 * 128 : (c0 + k) * 128]
            idx_blocks.append(flat.reshape(-1, 16).T)
        gbase += g
    idx = np.tile(np.concatenate(idx_blocks, axis=1), (8, 1))
    return idx, np.ascontiguousarray(drel), np.ascontiguousarray(wgt)


def _build_program(nwin, g_slot, gA_slot, gB_slot, idx_cols, n_cores=N_CORES):
    from contextlib import ExitStack

    import concourse.tile as tile
    from concourse import bacc, mybir

    f32 = mybir.dt.float32
    gdt = mybir.dt.float16 if GDTYPE == "f16" else mybir.dt.float32
    i16 = mybir.dt.int16

    nc = bacc.Bacc(
        "TRN2", target_bir_lowering=False, debug=False, num_devices=n_cores,
    )

    npad = nwin * WIN
    tot_g = int(g_slot.sum())

    h_t = nc.dram_tensor("h_src", [N_NODES, D], gdt, kind="ExternalInput")
    idx_t = nc.dram_tensor("idx", [128, idx_cols], i16, kind="ExternalInput")
    drel_t = nc.dram_tensor("drel", [128, tot_g], f32, kind="ExternalInput")
    wgt_t = nc.dram_tensor("wgt", [128, tot_g], f32, kind="ExternalInput")
    iota_t = nc.dram_tensor("iota", [128, 128], gdt, kind="ExternalInput")
    w_t = nc.dram_tensor("wmat", [D, D], gdt, kind="ExternalInput")
    b_t = nc.dram_tensor("bcol", [D, 1], f32, kind="ExternalInput")
    out_t = nc.dram_tensor("outT", [D, npad], f32, kind="ExternalOutput")

    with tile.TileContext(nc) as tc:
        with ExitStack() as ctx:
            const = ctx.enter_context(tc.tile_pool(name="const", bufs=1))
            gpool = ctx.enter_context(tc.tile_pool(name="gather", bufs=6))
            spool = ctx.enter_context(tc.tile_pool(name="sel", bufs=3))
            opool = ctx.enter_context(tc.tile_pool(name="outsb", bufs=2))
            ps_agg = ctx.enter_context(tc.tile_pool(name="ps_agg", bufs=2, space="PSUM"))
            ps_out = ctx.enter_context(tc.tile_pool(name="ps_out", bufs=2, space="PSUM"))

            idx = const.tile(list(idx_t.shape), i16)
            drel = const.tile(list(drel_t.shape), f32)
            wgt = const.tile(list(wgt_t.shape), f32)
            iota = const.tile([128, 128], gdt)
            wmat = const.tile([D, D], gdt)
            bcol = const.tile([D, 1], f32)
            agg_all = const.tile([128, npad], gdt, tag="agg_all")

            for sb, dr in ((idx, idx_t), (drel, drel_t), (wgt, wgt_t),
                           (iota, iota_t), (wmat, w_t), (bcol, b_t)):
                nc.sync.dma_start(sb[:], dr[:])

            h_A = h_t[0:IMG_A_ROWS, :]
            h_B = h_t[IMG_B_BASE:N_NODES, :]

            col = 0    # idx column cursor (units of 8 cols per group)
            gbase = 0  # group column cursor
            done_slots = 0
            next_t0 = 0
            CH = 512

            def emit_transform(t0, n):
                po = ps_out.tile([128, CH], f32, tag="psout")
                nc.tensor.matmul(
                    po[:, :n], wmat[:], agg_all[:, t0 : t0 + n],
                    start=True, stop=True,
                )
                ob = opool.tile([128, CH], f32, tag="outsb")
                nc.scalar.add(ob[:, :n], po[:, :n], bcol[:])
                nc.sync.dma_start(out_t[:, t0 : t0 + n], ob[:, :n])

            for slot in range(nwin):
                gA, gB, g = int(gA_slot[slot]), int(gB_slot[slot]), int(g_slot[slot])
                gtiles = []
                for img, gimg in ((h_A, gA), (h_B, gB)):
                    for (c0, k) in _chunks(gimg):
                        gt = gpool.tile([128, k, 128], gdt, tag="g")
                        nc.gpsimd.dma_gather(
                            gt[:], img, idx[:, col : col + k * 8],
                            num_idxs=k * 128, num_idxs_reg=k * 128, elem_size=D,
                        )
                        col += k * 8
                        gtiles.append((gt, k))

                # S: one fused tensor_scalar per group:
                # s[:, j, :] = (iota == drel[:, gbase+j]) * wgt[:, gbase+j]
                s = spool.tile([128, g, 128], gdt, tag="sel")
                for j in range(g):
                    nc.vector.tensor_scalar(
                        s[:, j, :], iota[:],
                        drel[:, gbase + j : gbase + j + 1],
                        wgt[:, gbase + j : gbase + j + 1],
                        mybir.AluOpType.is_equal, mybir.AluOpType.mult,
                    )

                psum = ps_agg.tile([128, 128], f32, tag="psagg")
                gi = 0
                for (gt, k) in gtiles:
                    for j in range(k):
                        nc.tensor.matmul(
                            psum[:], gt[:, j, :], s[:, gi, :],
                            start=(gi == 0), stop=(gi == g - 1),
                        )
                        gi += 1
                nc.scalar.copy(agg_all[:, slot * WIN : (slot + 1) * WIN], psum[:])
                gbase += g
                done_slots += 1
                # transform any complete 512-col chunk whose slots are done
                while done_slots * WIN >= next_t0 + CH:
                    emit_transform(next_t0, CH)
                    next_t0 += CH

            while next_t0 < npad:
                n = min(CH, npad - next_t0)
                emit_transform(next_t0, n)
                next_t0 += n

    nc.compile()
    return nc


def kernel(H, edge_src, edge_dst, edge_weight, W, b):
    global LAST_EXEC_NS, LAST_RESULTS
    from concourse import bass_utils

    H = np.asarray(H, dtype=np.float32)
    edge_src = np.asarray(edge_src, dtype=np.int32)
    edge_dst = np.asarray(edge_dst, dtype=np.int32)
    edge_weight = np.asarray(edge_weight, dtype=np.float32)
    W = np.asarray(W, dtype=np.float32)
    b = np.asarray(b, dtype=np.float32)

    np_g = np.float16 if GDTYPE == "f16" else np.float32
    cores, order, nwin, g_slot, gA_slot, gB_slot = _prep(edge_src, edge_dst)

    h_src = np.ascontiguousarray(H.astype(np_g))
    iota = np.tile(np.arange(128, dtype=np_g), (128, 1))
    wmat = np.ascontiguousarray(W.astype(np_g))
    bcol = np.ascontiguousarray(b.astype(np.float32).reshape(D, 1))
    in_maps = []
    idx_cols = None
    for c in range(N_CORES):
        idx, drel, wgt = _pack_core(
            cores[c], order[c], nwin, g_slot, gA_slot, gB_slot, edge_weight,
            np_g,
        )
        idx_cols = idx.shape[1]
        in_maps.append({
            "h_src": h_src, "idx": idx, "drel": drel, "wgt": wgt,
            "iota": iota, "wmat": wmat, "bcol": bcol,
        })

    nc = _build_program(nwin, g_slot, gA_slot, gB_slot, idx_cols)

    if os.environ.get("GCN_SIM", "0") == "1":
        from concourse.bass_interp import CoreSim

        out = np.empty((N_NODES, D), np.float32)
        for c in range(N_CORES):
            sim = CoreSim(nc)
            for k2, v2 in in_maps[c].items():
                sim.tensor(k2)[:] = v2
            sim.simulate()
            outT = np.array(sim.tensor("outT"))
            # slot i columns -> window order[c][i]
            for slot in range(nwin):
                wi = int(order[c][slot])
                w0, w1 = wi * WIN, min((wi + 1) * WIN, NPW)
                out[c * NPW + w0 : c * NPW + w1, :] = (
                    outT[:, slot * WIN : slot * WIN + (w1 - w0)].T)
        return out

    trace = os.environ.get("GCN_TRACE", "0") == "1"
    kw = {}
    if trace:
        import shutil
        td = "/tmp/gcn_ntff"
        shutil.rmtree(td, ignore_errors=True)
        os.makedirs(td, exist_ok=True)
        kw["tmpdir"] = td
    import time as _time
    last_err = None
    for backoff in (15, 45, 90, 0):
        try:
            res = bass_utils.run_bass_kernel_spmd(
                nc, in_maps, core_ids=list(range(N_CORES)), trace=trace, **kw
            )
            break
        except Exception as e:
            last_err = e
            if backoff:
                _time.sleep(backoff)
    else:
        raise last_err
    LAST_EXEC_NS = res.exec_time_ns
    LAST_RESULTS = res

    out = np.empty((N_NODES, D), np.float32)
    for c in range(N_CORES):
        outT = res.results[c]["outT"]
        for slot in range(nwin):
            wi = int(order[c][slot])
            w0, w1 = wi * WIN, min((wi + 1) * WIN, NPW)
            out[c * NPW + w0 : c * NPW + w1, :] = (
                outT[:, slot * WIN : slot * WIN + (w1 - w0)].T)
    return out


# revision 12
# speedup vs baseline: 1.1749x; 1.1749x over previous
"""GCN layer (gather -> weighted scatter-sum -> dense transform) on 8 trn2 cores.

Strategy (1-D row partitioning of destination nodes), v2:
  - Core c owns destination nodes [c*NPW, (c+1)*NPW). edge_dst is sorted, so
    each core's edges are a contiguous slice of the edge list.
  - Dst nodes are processed in windows of 128 (PSUM partition size). Each core
    processes ITS windows in DESCENDING edge-count order ("slots"), so the
    shared SPMD per-slot budgets (max over cores) track the per-core sorted
    quantiles and padding stays ~2%. The host unpermutes output columns.
  - Gather indices are int16 for SWDGE dma_gather. Instead of a hard lo/hi
    split at 32768 (which pads two streams separately), we gather from two
    OVERLAPPING views of H: image A = rows [0, 32768), image B = rows
    [17232, 50000) (both int16-addressable). Edges with src in the overlap
    [17232, 32768) are flexible and are assigned per-core to fill image-A
    groups exactly, so each slot needs only ceil(max_edges/128) total groups
    (single rounding).
  - Per 128-edge group: TensorE accumulates aggT[feat, dst] += G.T @ S in
    PSUM, where G = dma_gather'd source rows (fp16) and S[e, dst] =
    (iota[dst] == drel[e]) * w[e] built by ONE fused DVE tensor_scalar per
    group (two per-partition scalar operands) -- half the DVE traffic of the
    two-pass tensor_tensor build, which also reduces SBUF-port contention
    with the Q7 SWDGE descriptor writes.
  - The final transform out.T = W.T @ aggT (+b) is interleaved into the
    window loop (one 512-col chunk per 4 finished slots) so it hides under
    the gather stream instead of adding tail latency.
  - dma_gather calls are chunks of <=8 groups (8k+1 SWDGE ring entries;
    k<=8 proven safe on HW, k=12 crashes the exec unit).
"""

import os
import numpy as np

N_CORES = 8
N_NODES = 50000
D = 128
NPW = N_NODES // N_CORES  # 6250 dst nodes per core
WIN = 128
IMG_A_ROWS = 32768       # image A = H[0:32768]
CUT_B = 32768            # src >= CUT_B must use image B
IMG_B_BASE = N_NODES - 32768  # 17232; image B = H[17232:50000]
MAXG = 8                 # max groups (128 idx each) per dma_gather call

GDTYPE = os.environ.get("GCN_GDTYPE", "f16")

LAST_EXEC_NS = None
LAST_RESULTS = None


def _ceil_div(a, b):
    return -(-a // b)


def _prep(edge_src, edge_dst):
    """Host-side plan: per-core slot-ordered windows, shared slot budgets,
    per-core packed edge lists (A-image then B-image groups)."""
    nwin = _ceil_div(NPW, WIN)
    cores = []
    cnt = np.zeros((N_CORES, nwin), np.int64)
    cntA = np.zeros((N_CORES, nwin), np.int64)  # must-A (src < IMG_B_BASE)
    cntB = np.zeros((N_CORES, nwin), np.int64)  # must-B (src >= CUT_B)
    for c in range(N_CORES):
        e0, e1 = np.searchsorted(edge_dst, [c * NPW, (c + 1) * NPW])
        d = edge_dst[e0:e1] - c * NPW
        s = edge_src[e0:e1]
        bounds = [np.searchsorted(d, wi * WIN) for wi in range(nwin + 1)]
        wins = []
        for wi in range(nwin):
            i0, i1 = bounds[wi], bounds[wi + 1]
            wins.append((s[i0:i1], d[i0:i1] - wi * WIN, (e0 + i0, e0 + i1)))
            cnt[c, wi] = i1 - i0
            cntA[c, wi] = int((s[i0:i1] < IMG_B_BASE).sum())
            cntB[c, wi] = int((s[i0:i1] >= CUT_B).sum())
        cores.append(wins)

    order = np.argsort(-cnt, axis=1)  # per-core slot -> window
    cs = np.take_along_axis(cnt, order, 1)
    As = np.take_along_axis(cntA, order, 1)
    Bs = np.take_along_axis(cntB, order, 1)

    g_slot = np.maximum(
        _ceil_div(cs.max(0), 128),
        _ceil_div(As.max(0), 128) + _ceil_div(Bs.max(0), 128),
    ).astype(int)
    g_slot = np.maximum(g_slot, 1)
    gA_lo = np.maximum(_ceil_div(As.max(0), 128), 0).astype(int)
    gA_hi = (g_slot - np.maximum(_ceil_div(Bs.max(0), 128), 0)).astype(int)
    # prefer gA a multiple of MAXG (min call count), else low end
    gA_slot = np.empty(nwin, int)
    for i in range(nwin):
        lo, hi = int(gA_lo[i]), int(gA_hi[i])
        lo = max(lo, 0)
        hi = max(hi, lo)
        mult = _ceil_div(lo, MAXG) * MAXG
        gA_slot[i] = mult if lo <= mult <= hi else lo
    gB_slot = g_slot - gA_slot
    return cores, order, nwin, g_slot, gA_slot, gB_slot


def _chunks(g):
    out = []
    c0 = 0
    while c0 < g:
        k = min(MAXG, g - c0)
        out.append((c0, k))
        c0 += k
    return out


def _pack_core(wins, order_c, nwin, g_slot, gA_slot, gB_slot, edge_weight,
               np_g):
    np_m = np_g  # metadata dtype matches gather dtype (tensor_tensor S build)
    """Build device arrays for one core.

    idx: per-call wrapped-16 blocks, concatenated: [16, sum(call 8*ke)] -> tiled
         to [128, .].
    drel/wgt: [128, sum(g_slot)] with column (slot base + group) and row p =
         edge at group position p."""
    tot_g = int(g_slot.sum())
    drel = np.zeros((128, tot_g), np_m)
    wgt = np.zeros((128, tot_g), np_m)
    idx_blocks = []
    gbase = 0
    for slot in range(nwin):
        wi = int(order_c[slot])
        s, d, (ge0, ge1) = wins[wi]
        w = edge_weight[ge0:ge1]
        gA, gB, g = int(gA_slot[slot]), int(gB_slot[slot]), int(g_slot[slot])
        capA = 128 * gA
        isB_forced = s >= CUT_B
        isA_forced = s < IMG_B_BASE
        nA0 = int(isA_forced.sum())
        # fill A with forced-A then flex until capA
        flex = ~isA_forced & ~isB_forced
        takeA_flex = min(max(capA - nA0, 0), int(flex.sum()))
        # order: forced-A edges, then takeA_flex flex edges -> image A
        idxA = np.flatnonzero(isA_forced)
        idxF = np.flatnonzero(flex)
        idxB = np.flatnonzero(isB_forced)
        selA = np.concatenate([idxA, idxF[:takeA_flex]])
        selB = np.concatenate([idxF[takeA_flex:], idxB])
        assert len(selA) <= 128 * gA and len(selB) <= 128 * gB, (
            slot, len(selA), gA, len(selB), gB)
        iA = np.zeros(128 * gA, np.int16)
        iB = np.zeros(128 * gB, np.int16)
        iA[: len(selA)] = s[selA].astype(np.int16)
        iB[: len(selB)] = (s[selB] - IMG_B_BASE).astype(np.int16)
        # metadata per group-column; A groups then B groups
        dd = np.zeros(128 * g, np_m)
        ww = np.zeros(128 * g, np_m)
        dd[: len(selA)] = d[selA].astype(np_m)
        ww[: len(selA)] = w[selA].astype(np_g).astype(np_m)
        dd[128 * gA : 128 * gA + len(selB)] = d[selB].astype(np_m)
        ww[128 * gA : 128 * gA + len(selB)] = w[selB].astype(np_g).astype(np_m)
        drel[:, gbase : gbase + g] = dd.reshape(g, 128).T
        wgt[:, gbase : gbase + g] = ww.reshape(g, 128).T
        # idx blocks per call (A calls then B calls), wrapped 16
        for (c0, k) in _chunks(gA):
            flat = iA[c0 * 128 : (c0 + k) * 128]
            idx_blocks.append(flat.reshape(-1, 16).T)
        for (c0, k) in _chunks(gB):
            flat = iB[c0 * 128 : (c0 + k) * 128]
            idx_blocks.append(flat.reshape(-1, 16).T)
        gbase += g
    idx = np.tile(np.concatenate(idx_blocks, axis=1), (8, 1))
    return idx, np.ascontiguousarray(drel), np.ascontiguousarray(wgt)


def _build_program(nwin, g_slot, gA_slot, gB_slot, idx_cols, n_cores=N_CORES):
    from contextlib import ExitStack

    import concourse.tile as tile
    from concourse import bacc, mybir

    f32 = mybir.dt.float32
    gdt = mybir.dt.float16 if GDTYPE == "f16" else mybir.dt.float32
    i16 = mybir.dt.int16

    nc = bacc.Bacc(
        "TRN2", target_bir_lowering=False, debug=False, num_devices=n_cores,
    )

    npad = nwin * WIN
    tot_g = int(g_slot.sum())

    h_t = nc.dram_tensor("h_src", [N_NODES, D], gdt, kind="ExternalInput")
    idx_t = nc.dram_tensor("idx", [128, idx_cols], i16, kind="ExternalInput")
    drel_t = nc.dram_tensor("drel", [128, tot_g], gdt, kind="ExternalInput")
    wgt_t = nc.dram_tensor("wgt", [128, tot_g], gdt, kind="ExternalInput")
    iota_t = nc.dram_tensor("iota", [128, 128], gdt, kind="ExternalInput")
    w_t = nc.dram_tensor("wmat", [D, D], gdt, kind="ExternalInput")
    b_t = nc.dram_tensor("bcol", [D, 1], f32, kind="ExternalInput")
    out_t = nc.dram_tensor("outT", [D, npad], f32, kind="ExternalOutput")

    with tile.TileContext(nc) as tc:
        with ExitStack() as ctx:
            const = ctx.enter_context(tc.tile_pool(name="const", bufs=1))
            gpool = ctx.enter_context(tc.tile_pool(name="gather", bufs=6))
            spool = ctx.enter_context(tc.tile_pool(name="sel", bufs=3))
            opool = ctx.enter_context(tc.tile_pool(name="outsb", bufs=2))
            ps_agg = ctx.enter_context(tc.tile_pool(name="ps_agg", bufs=2, space="PSUM"))
            ps_out = ctx.enter_context(tc.tile_pool(name="ps_out", bufs=2, space="PSUM"))

            idx = const.tile(list(idx_t.shape), i16)
            drel = const.tile(list(drel_t.shape), gdt)
            wgt = const.tile(list(wgt_t.shape), gdt)
            iota = const.tile([128, 128], gdt)
            wmat = const.tile([D, D], gdt)
            bcol = const.tile([D, 1], f32)
            agg_all = const.tile([128, npad], gdt, tag="agg_all")

            for sb, dr in ((idx, idx_t), (drel, drel_t), (wgt, wgt_t),
                           (iota, iota_t), (wmat, w_t), (bcol, b_t)):
                nc.sync.dma_start(sb[:], dr[:])

            h_A = h_t[0:IMG_A_ROWS, :]
            h_B = h_t[IMG_B_BASE:N_NODES, :]

            col = 0    # idx column cursor (units of 8 cols per group)
            gbase = 0  # group column cursor
            done_slots = 0
            next_t0 = 0
            CH = 512

            def emit_transform(t0, n):
                po = ps_out.tile([128, CH], f32, tag="psout")
                nc.tensor.matmul(
                    po[:, :n], wmat[:], agg_all[:, t0 : t0 + n],
                    start=True, stop=True,
                )
                ob = opool.tile([128, CH], f32, tag="outsb")
                nc.scalar.add(ob[:, :n], po[:, :n], bcol[:])
                nc.sync.dma_start(out_t[:, t0 : t0 + n], ob[:, :n])

            for slot in range(nwin):
                gA, gB, g = int(gA_slot[slot]), int(gB_slot[slot]), int(g_slot[slot])
                gtiles = []
                for img, gimg in ((h_A, gA), (h_B, gB)):
                    for (c0, k) in _chunks(gimg):
                        gt = gpool.tile([128, k, 128], gdt, tag="g")
                        nc.gpsimd.dma_gather(
                            gt[:], img, idx[:, col : col + k * 8],
                            num_idxs=k * 128, num_idxs_reg=k * 128, elem_size=D,
                        )
                        col += k * 8
                        gtiles.append((gt, k))

                # S for the whole slot in 2 DVE ops via step-0 broadcast APs:
                # s[p, j, n] = (iota[n] == drel[p, gbase+j]) * wgt[p, gbase+j]
                s = spool.tile([128, g, 128], gdt, tag="sel")
                sh = (128, g, 128)
                if os.environ.get("GCN_SPROBE", "0") == "1":
                    # perf probe: no DVE S-build (output is wrong)
                    nc.vector.memset(s[:], 0)
                else:
                    nc.vector.tensor_tensor(
                        s[:], iota[:, None, :].broadcast_to(sh),
                        drel[:, gbase : gbase + g, None].broadcast_to(sh),
                        mybir.AluOpType.is_equal,
                    )
                    nc.vector.tensor_tensor(
                        s[:], s[:], wgt[:, gbase : gbase + g, None].broadcast_to(sh),
                        mybir.AluOpType.mult,
                    )

                psum = ps_agg.tile([128, 128], f32, tag="psagg")
                gi = 0
                for (gt, k) in gtiles:
                    for j in range(k):
                        nc.tensor.matmul(
                            psum[:], gt[:, j, :], s[:, gi, :],
                            start=(gi == 0), stop=(gi == g - 1),
                        )
                        gi += 1
                nc.scalar.copy(agg_all[:, slot * WIN : (slot + 1) * WIN], psum[:])
                gbase += g
                done_slots += 1
                # transform any complete 512-col chunk whose slots are done
                while done_slots * WIN >= next_t0 + CH:
                    emit_transform(next_t0, CH)
                    next_t0 += CH

            while next_t0 < npad:
                n = min(CH, npad - next_t0)
                emit_transform(next_t0, n)
                next_t0 += n

    nc.compile()
    return nc


def kernel(H, edge_src, edge_dst, edge_weight, W, b):
    global LAST_EXEC_NS, LAST_RESULTS
    from concourse import bass_utils

    H = np.asarray(H, dtype=np.float32)
    edge_src = np.asarray(edge_src, dtype=np.int32)
    edge_dst = np.asarray(edge_dst, dtype=np.int32)
    edge_weight = np.asarray(edge_weight, dtype=np.float32)
    W = np.asarray(W, dtype=np.float32)
    b = np.asarray(b, dtype=np.float32)

    np_g = np.float16 if GDTYPE == "f16" else np.float32
    cores, order, nwin, g_slot, gA_slot, gB_slot = _prep(edge_src, edge_dst)

    h_src = np.ascontiguousarray(H.astype(np_g))
    iota = np.tile(np.arange(128, dtype=np_g), (128, 1))
    wmat = np.ascontiguousarray(W.astype(np_g))
    bcol = np.ascontiguousarray(b.astype(np.float32).reshape(D, 1))
    in_maps = []
    idx_cols = None
    for c in range(N_CORES):
        idx, drel, wgt = _pack_core(
            cores[c], order[c], nwin, g_slot, gA_slot, gB_slot, edge_weight,
            np_g,
        )
        idx_cols = idx.shape[1]
        in_maps.append({
            "h_src": h_src, "idx": idx, "drel": drel, "wgt": wgt,
            "iota": iota, "wmat": wmat, "bcol": bcol,
        })

    nc = _build_program(nwin, g_slot, gA_slot, gB_slot, idx_cols)

    if os.environ.get("GCN_SIM", "0") == "1":
        from concourse.bass_interp import CoreSim

        out = np.empty((N_NODES, D), np.float32)
        for c in range(N_CORES):
            sim = CoreSim(nc)
            for k2, v2 in in_maps[c].items():
                sim.tensor(k2)[:] = v2
            sim.simulate()
            outT = np.array(sim.tensor("outT"))
            # slot i columns -> window order[c][i]
            for slot in range(nwin):
                wi = int(order[c][slot])
                w0, w1 = wi * WIN, min((wi + 1) * WIN, NPW)
                out[c * NPW + w0 : c * NPW + w1, :] = (
                    outT[:, slot * WIN : slot * WIN + (w1 - w0)].T)
        return out

    trace = os.environ.get("GCN_TRACE", "0") == "1"
    kw = {}
    if trace:
        import shutil
        td = "/tmp/gcn_ntff"
        shutil.rmtree(td, ignore_errors=True)
        os.makedirs(td, exist_ok=True)
        kw["tmpdir"] = td
    import time as _time
    last_err = None
    for backoff in (15, 45, 90, 0):
        try:
            res = bass_utils.run_bass_kernel_spmd(
                nc, in_maps, core_ids=list(range(N_CORES)), trace=trace, **kw
            )
            break
        except Exception as e:
            last_err = e
            if backoff:
                _time.sleep(backoff)
    else:
        raise last_err
    LAST_EXEC_NS = res.exec_time_ns
    LAST_RESULTS = res

    out = np.empty((N_NODES, D), np.float32)
    for c in range(N_CORES):
        outT = res.results[c]["outT"]
        for slot in range(nwin):
            wi = int(order[c][slot])
            w0, w1 = wi * WIN, min((wi + 1) * WIN, NPW)
            out[c * NPW + w0 : c * NPW + w1, :] = (
                outT[:, slot * WIN : slot * WIN + (w1 - w0)].T)
    return out
